# revision 44
# baseline (speedup 1.0000x reference)
"""Trainium2 Bass kernel for nn_EvenLayer (LDPC min-sum check-node update).

Reference semantics (B=8 batches, E=3600 edges):
    neighbor = inf_mask == 0            # (E, E)
    signs    = sign(prod(where(neighbor, x, 1), axis=-1))
    mins     = min(|x| + inf_mask, axis=-1)
    out      = signs * max(mins - bias, 0)

The mask encodes "shares a check node, excluding self" — an equivalence
relation minus the diagonal. The host verifies that structure at runtime
(values only {0, +inf}, empty diagonal, rows = leader-equality classes);
on success each edge-group (check node, size d=6) is packed into slots,
sharded over the 8 cores, and a small SPMD kernel computes per slot the
leave-one-out min of |x| and the leave-one-out product sign, bit-exact vs
the reference. If verification fails, a generic dense kernel computes the
masked reductions directly from the mask data.

The d=6 fast path (5988ns baseline -> 1057ns) avoids the ~2.2us fixed
HWDGE/DGE/semaphore latency of plain dma_start in BOTH directions by
routing the input and output through the Pool-engine SWDGE
prepare/trigger ring (dma_gather in / kv_writeback out): descriptors are
generated during otherwise-dead time and each transfer is fired by a
cheap trigger_dma. Compute is split by engine ALU support — the DVE does
abs / the 3-op LOO-min (duplicated pair-min trick) / bias-relu / the
fused (sign & 0x80000000) | relu combine, while the Pool engine computes
the LOO product (sign carrier, mult-only) plus all descriptor preps,
ucode library switches, and triggers. The block-exit all-engine barrier
is elided (engines drain independently; the trigger's own engine fences
the output ring).
"""

import numpy as np

B, E, NCORES = 8, 3600, 8

_NC_CACHE = {}
TRACE = False
LAST_RESULT = None  # BassKernelResults of the last run (for test harness)


def _analyze(inf_mask):
    """Return leader labels if the mask is exactly an equivalence relation
    minus the diagonal with values {0, +inf}; else None."""
    m = np.asarray(inf_mask)
    if m.ndim != 2 or m.shape[0] != m.shape[1]:
        return None
    if not np.all((m == 0) | np.isposinf(m)):
        return None
    nb = m == 0
    if nb.diagonal().any():
        return None
    n = m.shape[0]
    idx = np.arange(n)
    first = np.argmax(nb, axis=1)
    has = nb.any(axis=1)
    leader = np.where(has, np.minimum(idx, first), idx)
    eq = leader[:, None] == leader[None, :]
    np.fill_diagonal(eq, False)
    if not np.array_equal(nb, eq):
        return None
    return leader


def _build_slots(leader, nbatch=B):
    """Pack groups into (NCORES, blocks, gpb, d) slot->edge index array (-1 pad)."""
    max_blocks = max(128 // nbatch, 1)
    order = np.argsort(leader, kind="stable")
    lead_sorted = leader[order]
    uniq, counts = np.unique(lead_sorted, return_counts=True)
    G = len(uniq)
    d = max(int(counts.max()), 2)
    G8 = ((G + NCORES - 1) // NCORES) * NCORES
    slot_edge = np.full((G8, d), -1, dtype=np.int64)
    col = np.concatenate([np.arange(c) for c in counts])
    row = np.repeat(np.arange(G), counts)
    slot_edge[row, col] = order
    Gc = G8 // NCORES
    gpb = (Gc + max_blocks - 1) // max_blocks   # groups per partition-block
    blocks = (Gc + gpb - 1) // gpb
    Gcp = blocks * gpb
    slot_all = slot_edge.reshape(NCORES, Gc, d)
    if Gcp != Gc:
        pad = np.full((NCORES, Gcp - Gc, d), -1, dtype=np.int64)
        slot_all = np.concatenate([slot_all, pad], axis=1)
    return slot_all.reshape(NCORES, blocks, gpb, d), d, blocks, gpb


def _build_fast_nc(P, F, gpb, d):
    """Raw-bass kernel (no TileContext — the walrus in this container rejects
    instructions carrying >2 sync waits, which Tile's tail drain emits).

    Input "xb" packs [x_slots | bias_slots] as (P, 2F); output "ys" is (P, F).
    Per slot s of each group g (slots along the innermost dim, d per group):
        A    = |x|                               (ACT, parallel with DVE)
        GP_g = prod_s x[g,s]                     (raw product; +inf pads are
                                                  sign-neutral)
        T    = GP_g * x  -> sign(T) = sign of leave-one-out product (x^2 > 0)
        M    = leave-one-out min of A via fused prefix/suffix chain:
               Wbuf[c] = (pre[c], suf[d-1-c]) pairs, one TT-min per step
        out  = (relu(M - bias)) | signbit(T)     (bitwise or; relu >= 0)
    """
    import contextlib

    import concourse.bass as bass
    from concourse import mybir

    f32 = mybir.dt.float32
    i32 = mybir.dt.int32
    AL = mybir.AluOpType
    AX = mybir.AxisListType

    nc = bass.Bass()
    xb = nc.declare_dram_parameter("xb", [P, 2 * F], f32, isOutput=False)
    ys = nc.declare_dram_parameter("ys", [P, F], f32, isOutput=True)

    with contextlib.ExitStack() as ctx:
        XB = ctx.enter_context(nc.sbuf_tensor("XB", [P, 2 * F], f32))
        A = ctx.enter_context(nc.sbuf_tensor("A", [P, F], f32))
        T = ctx.enter_context(nc.sbuf_tensor("T", [P, F], f32))
        Ti = ctx.enter_context(nc.sbuf_tensor("Ti", [P, F], i32))
        Km = ctx.enter_context(nc.sbuf_tensor("Km", [P, F], i32))
        Kp = ctx.enter_context(nc.sbuf_tensor("Kp", [P, F], i32))
        M = ctx.enter_context(nc.sbuf_tensor("M", [P, F], f32))
        Wb = ctx.enter_context(nc.sbuf_tensor("Wb", [P, gpb, max(d - 2, 1), 2], f32))
        Wp = ctx.enter_context(nc.sbuf_tensor("Wp", [P, gpb, max(d - 2, 1), 2], f32))
        R = ctx.enter_context(nc.sbuf_tensor("R", [P, F], f32))
        O = ctx.enter_context(nc.sbuf_tensor("O", [P, F], i32))

        s_in = ctx.enter_context(nc.semaphore("s_in"))
        s_dve = ctx.enter_context(nc.semaphore("s_dve"))
        s_out = ctx.enter_context(nc.semaphore("s_out"))
        s_v = ctx.enter_context(nc.semaphore("s_v"))
        block = ctx.enter_context(nc.Block())

        X = XB[:, 0:F]
        Bt = XB[:, F : 2 * F]

        @block.sync
        def _(sync):
            sync.dma_start(out=XB[:], in_=xb[:]).then_inc(s_in, 16)
            sync.wait_ge(s_dve, 1)
            sync.dma_start(out=ys[:], in_=O[:].bitcast(f32)).then_inc(s_out, 16)
            sync.wait_ge(s_out, 16)

        @block.vector
        def _(vector):
            X3 = X.rearrange("p (g d) -> p g d", d=d)
            A3 = A[:].rearrange("p (g d) -> p g d", d=d)
            M3 = M[:].rearrange("p (g d) -> p g d", d=d)
            T3 = T[:].rearrange("p (g d) -> p g d", d=d)

            # DVE self-sem chain: every op incs s_v; dependent ops wait on the
            # producer's count (same-engine RAW through SBUF needs sync).
            # A high-water mark elides waits already covered. (Attaching
            # waits to sync_info instead was tested: identical timing — the
            # sequencer pipelines wait decode behind op execution.)
            cnt = [0]
            waited = [0]

            def emit(fn, wait=None):
                if wait is None:
                    wait = cnt[0]          # default: wait for all prior DVE ops
                if wait > waited[0]:
                    vector.wait_ge(s_v, wait)
                    waited[0] = wait
                fn().then_inc(s_v, 1)
                cnt[0] += 1
                return cnt[0]              # sem value once this op completes

            def tt(out, a, b, op, wait=None):
                return emit(
                    lambda: nc.vector.tensor_tensor(out=out, in0=a, in1=b, op=op),
                    wait=wait,
                )

            def loo_chain(src_h, src3, out_h, out3, wb_h, op, first_wait):
                """Leave-one-out reduction of `op` over the d slots of each
                group. d==6/d==4 use a tournament tree (4 / 2 ops); other d
                use a fused prefix/suffix pair chain (d ops)."""
                soff = src3.offset
                pstep, gstep = src3.ap[0], src3.ap[1]
                ooff = out3.offset
                opp, opg = out3.ap[0], out3.ap[1]

                def sv(off, apdims):  # view into src
                    return bass.AP(src_h, soff + off, [pstep, gstep] + apdims)

                if d == 2:
                    emit(
                        lambda: nc.vector.tensor_copy(
                            out3, sv(1, [[-1, 2]])
                        ),
                        wait=first_wait,
                    )
                    return
                if d == 4:
                    # mp[k] = op(A[2k], A[2k+1]); out[2k+s] = op(A[2k+1-s], mp[1-k])
                    t0 = tt(wb_h[:, :, 0, :], sv(0, [[2, 2]]), sv(1, [[2, 2]]), op,
                            wait=first_wait)
                    wb4 = wb_h[:, :, :, :]
                    mp_swap_b = bass.AP(wb_h, wb4.offset + 1, [wb4.ap[0], wb4.ap[1], [-1, 2], [0, 2]])
                    tt(bass.AP(out_h, ooff, [opp, opg, [2, 2], [1, 2]]),
                       sv(1, [[2, 2], [-1, 2]]), mp_swap_b, op, wait=t0)
                    return
                if d == 6:
                    # wb flat view: 8 contiguous slots per group; use 0..5
                    wb4 = wb_h[:, :, :, :]
                    wboff = wb4.offset
                    wv = lambda off, apdims: bass.AP(wb_h, wboff + off, [wb4.ap[0], wb4.ap[1]] + apdims)
                    # L1: mp[k] = op(A[2k], A[2k+1]), k=0..2 -> wb slots 0..2
                    t0 = tt(wv(0, [[1, 3]]), sv(0, [[2, 3]]), sv(1, [[2, 3]]), op,
                            wait=first_wait)
                    # L2: c0 = op(mp1, mp2), c1 = op(mp0, mp2) -> wb slots 3,4
                    tt(wv(3, [[1, 2]]), wv(1, [[-1, 2]]), wv(2, [[0, 2]]), op, wait=t0)
                    # c2 = op(mp0, mp1) -> wb slot 5
                    t2 = tt(wv(5, [[1, 1]]), wv(0, [[1, 1]]), wv(1, [[1, 1]]), op, wait=t0)
                    # L3: out[2k+s] = op(A[2k+1-s], c[k])
                    tt(bass.AP(out_h, ooff, [opp, opg, [2, 3], [1, 2]]),
                       sv(1, [[2, 3], [-1, 2]]), wv(3, [[1, 3], [0, 2]]), op, wait=t2)
                    return

                # generic: fused prefix/suffix pair chain
                def U(k):  # src slots (k, d-1-k); step may be negative
                    return sv(k, [[d - 1 - 2 * k, 2]])

                wb4 = wb_h[:, :, :, :]
                prev_t = emit(
                    lambda: nc.vector.tensor_copy(wb_h[:, :, 0, :], U(0)),
                    wait=first_wait,
                )
                for k in range(1, d - 2):
                    prev_t = tt(wb_h[:, :, k, :], wb_h[:, :, k - 1, :], U(k), op, wait=prev_t)
                # final chain step writes out[d-1] (pre[d-2]) and out[0] (suf[1])
                ends = bass.AP(out_h, ooff + d - 1, [opp, opg, [-(d - 1), 2]])
                tt(ends, wb_h[:, :, d - 3, :], U(d - 2), op, wait=prev_t)
                # middles: out[j] = pre[j-1] `op` suf[j+1], j = 1..d-2, one op
                pre_view = bass.AP(wb_h, wb4.offset, [wb4.ap[0], wb4.ap[1], [2, d - 2]])
                suf_rev = bass.AP(wb_h, wb4.offset + (d - 3) * 2 + 1, [wb4.ap[0], wb4.ap[1], [-2, d - 2]])
                tt(out3[:, :, 1 : d - 1], pre_view, suf_rev, op)

            # mask tiles (no data deps; run during the input DMA)
            emit(lambda: nc.vector.memset(Km[:], -2147483648), wait=0)
            t_msets = emit(lambda: nc.vector.memset(Kp[:], 2147483647), wait=0)

            vector.wait_ge(s_in, 16)
            # ---- |x| as one int32 AND (bit-exact abs, no scalar engine:
            # the first ACT activation pays a ~1.6us cold-table load) ----
            t_abs = emit(
                lambda: nc.vector.tensor_tensor(
                    out=A[:].bitcast(i32), in0=X.bitcast(i32), in1=Kp[:], op=AL.bitwise_and
                ),
                wait=t_msets,
            )
            # ---- leave-one-out product of raw x -> its sign bit (+inf pads
            # are positive, hence sign-neutral); interleaved with the min tree
            # so the wait high-water-mark elides the product tree's waits ----
            loo_chain(XB, X3, T, T3, Wp, AL.mult, first_wait=0)
            t_prod = cnt[0]
            loo_chain(A, A3, M, M3, Wb, AL.min, first_wait=t_abs)
            t_min = cnt[0]

            # ---- out = relu(M - bias) with the sign bit OR'd in ----
            t_sub = emit(lambda: nc.vector.tensor_sub(R[:], M[:], Bt), wait=t_min)
            emit(lambda: nc.vector.tensor_tensor(out=Ti[:], in0=T[:].bitcast(i32), in1=Km[:], op=AL.bitwise_and), wait=t_prod)
            emit(lambda: nc.vector.tensor_relu(out=R[:], in_=R[:]), wait=t_sub)
            vector.wait_ge(s_v, cnt[0])
            nc.vector.tensor_tensor(
                out=O[:], in0=R[:].bitcast(i32), in1=Ti[:], op=AL.bitwise_or
            ).then_inc(s_dve, 1)

    return nc


def _build_fast_nc6(P, F, gpb):
    """Optimized raw-bass kernel for d == 6 (the LDPC d_c=6 case).

    Both DMAs go through the Pool SWDGE prepare/trigger path (descriptor
    generation off the critical path; the trigger applies the transfer):
      * input:  dma_gather (mlp lib) with identity int16 indices from iota —
        xb rows are padded to 64 f32 (256B descriptor constraint) and the
        DRAM tensor to 240 rows (iota channel pattern upper bound).
      * output: kv_writeback (attn lib) with batch=F / ncn=1 so every AP
        keeps its partition dim first; ys lands transposed as (F, 128).
    Compute splits by engine ALU support (bitwise/min/compare are DVE-only;
    Pool TensorTensor does add/sub/mult):
      DVE:  A = x & 0x7fffffff; 3-op LOO-min (dup pair-mins W, LOO-of-3 C,
            final M); R = relu(M - bias); O = (Tx & 0x80000000) | R  (fused
            scalar_tensor_tensor; bit-exact vs signs*max(mins-bias, 0)).
      Pool: Tw/Tc/Tx — the LOO product (sign carrier) via the same 3-op
            structure with mult, plus all preps/triggers/library loads.
    """
    import contextlib
    import os

    import concourse.bass as bass
    from concourse import mybir

    f32 = mybir.dt.float32
    i16 = mybir.dt.int16
    i32 = mybir.dt.int32
    AL = mybir.AluOpType

    hwdge_in = bool(int(os.environ.get("K6_HWDGE_IN", "0")))
    # raw int16 iota as the gather idx table (no %16 fold): valid iff the HW
    # gather reads the table from partition block 0 only (like the sim).
    raw_iota = bool(int(os.environ.get("K6_RAW_IOTA", "0")))
    # split the idx fold: DVE does only the [128,1] AND, Pool (released
    # 100ns earlier) does the subtract + int16 convert next to the prep.
    pool_fold = bool(int(os.environ.get("K6_POOL_FOLD", "0")))
    assert P <= 128
    d = 6
    FP = 64                      # padded xb row (f32 elems): 256B descriptors
    ROWS = 240                   # iota idx upper bound (p + 16*s <= 239)

    nc = bass.Bass()
    xb = nc.declare_dram_parameter("xb", [ROWS, FP], f32, isOutput=False)
    ys = nc.declare_dram_parameter("ys", [F, 128], f32, isOutput=True)

    with contextlib.ExitStack() as ctx:
        XB = ctx.enter_context(nc.sbuf_tensor("XB", [128, FP], f32))
        A = ctx.enter_context(nc.sbuf_tensor("A", [P, F], f32))
        W = ctx.enter_context(nc.sbuf_tensor("W", [P, gpb, 6], f32))
        C = ctx.enter_context(nc.sbuf_tensor("C", [P, gpb, 3], f32))
        M = ctx.enter_context(nc.sbuf_tensor("M", [P, F], f32))
        R = ctx.enter_context(nc.sbuf_tensor("R", [P, F], f32))
        Tw = ctx.enter_context(nc.sbuf_tensor("Tw", [P, gpb, 6], f32))
        Tc = ctx.enter_context(nc.sbuf_tensor("Tc", [P, gpb, 3], f32))
        Tx = ctx.enter_context(nc.sbuf_tensor("Tx", [P, F], f32))
        O = ctx.enter_context(nc.sbuf_tensor("O", [128, F], f32))
        CI = ctx.enter_context(nc.sbuf_tensor("CI", [128, F], i32))
        IX = ctx.enter_context(nc.sbuf_tensor("IX", [128, 8], i16))
        IW = ctx.enter_context(nc.sbuf_tensor("IW", [128, 8], i32))
        IO = ctx.enter_context(nc.sbuf_tensor("IO", [128, 1], i32))
        KS = ctx.enter_context(nc.sbuf_tensor("KS", [P, 1], i32))

        s_in = ctx.enter_context(nc.semaphore("s_in"))
        s_v = ctx.enter_context(nc.semaphore("s_v"))
        s_p = ctx.enter_context(nc.semaphore("s_p"))
        s_prep = ctx.enter_context(nc.semaphore("s_prep"))
        s_dma = ctx.enter_context(nc.semaphore("s_dma"))
        # manual BassBlock: identical to nc.Block() but without the exit-time
        # all-engine barrier (~300ns); each engine queue drains on its own
        # and the SP wait on s_dma already fences the output DMA.
        block = bass.BassBlock(nc, f"block_{nc.next_id()}")
        nc.cur_block = block

        X = XB[0:P, 0:F]
        Bt = XB[0:P, F : 2 * F]

        def dup_views(view3, handle):
            off = view3.offset
            pdim, gdim = view3.ap[0], view3.ap[1]

            def V(extra, tail):
                return bass.AP(handle, off + extra, [pdim, gdim] + tail)

            return V

        if hwdge_in:
            @block.sync
            def _(sync):
                sync.dma_start(out=XB[:], in_=xb[0:128, :]).then_inc(s_in, 16)

        n_dve = [0]

        @block.vector
        def _(vector):
            A3 = A[:].rearrange("p (g d) -> p g d", d=d)
            M3 = M[:].rearrange("p (g d) -> p g d", d=d)
            AV = dup_views(A3, A)
            WV = dup_views(W[:], W)
            CV = dup_views(C[:], C)
            MV = dup_views(M3, M)

            cnt = [0]
            waited = [0]

            def emit(fn, wait=None):
                if wait is None:
                    wait = cnt[0]
                if wait > waited[0]:
                    vector.wait_ge(s_v, wait)
                    waited[0] = wait
                fn().then_inc(s_v, 1)
                cnt[0] += 1
                return cnt[0]

            def tt(out, a, b, op, wait=None):
                return emit(
                    lambda: nc.vector.tensor_tensor(out=out, in0=a, in1=b, op=op),
                    wait=wait,
                )

            # idx-table fold: IX[p, s] = (p %% 16) + 16*s from the Pool iota
            # IW (p + 16*s) — subtract p & ~15, cast to int16 (see Pool block)
            if not raw_iota and pool_fold:
                # only the bitwise AND here (DVE-only ALU); Pool finishes
                vector.wait_ge(s_p, 1)  # IW (Pool iota)
                emit(
                    lambda: nc.vector.tensor_single_scalar(
                        out=IO[:], in_=IW[:, 0:1], scalar=-16, op=AL.bitwise_and
                    ),
                    wait=0,
                )
                emit(lambda: nc.vector.engine_nop(), wait=0)
            elif not raw_iota:
                vector.wait_ge(s_p, 1)  # IW (Pool iota)
                t_off = emit(
                    lambda: nc.vector.tensor_single_scalar(
                        out=IO[:], in_=IW[:, 0:1], scalar=-16, op=AL.bitwise_and
                    ),
                    wait=0,
                )
                emit(
                    lambda: nc.vector.tensor_tensor(
                        out=IX[:],
                        in0=IW[:],
                        in1=bass.AP(IO, 0, [IO[:].ap[0], [0, 8]]),
                        op=AL.subtract,
                    ),
                    wait=t_off,
                )

            # dead-time setup: the sign-bit mask for the final combine
            emit(lambda: nc.vector.memset(KS[:], -0x80000000), wait=0)

            vector.wait_ge(s_in, 16)
            t_abs = emit(
                lambda: nc.vector.tensor_single_scalar(
                    out=A[:].bitcast(i32),
                    in_=X.bitcast(i32),
                    scalar=0x7FFFFFFF,
                    op=AL.bitwise_and,
                ),
                wait=0,
            )
            t_w = tt(WV(0, [[3, 2], [1, 3]]), AV(0, [[0, 2], [2, 3]]),
                     AV(1, [[0, 2], [2, 3]]), AL.min, wait=t_abs)
            t_c = tt(CV(0, [[1, 3]]), WV(1, [[1, 3]]), WV(2, [[1, 3]]),
                     AL.min, wait=t_w)
            t_m = tt(MV(0, [[2, 3], [1, 2]]), AV(1, [[2, 3], [-1, 2]]),
                     CV(0, [[1, 3], [0, 2]]), AL.min, wait=t_c)
            t_sub = emit(lambda: nc.vector.tensor_sub(R[:], M[:], Bt), wait=t_m)
            t_relu = emit(lambda: nc.vector.tensor_relu(out=R[:], in_=R[:]), wait=t_sub)
            # O = (Tx & signbit) | relu(M - bias); waits: own chain + Pool
            # (3 sign mults and the O-pad memset, all <= s_p high-water)
            vector.wait_ge(s_v, t_relu)
            waited[0] = t_relu
            vector.wait_ge(s_p, 8 if pool_fold else 6)
            nc.vector.scalar_tensor_tensor(
                out=O[0:P, :].bitcast(i32),
                in0=Tx[:].bitcast(i32),
                scalar=KS[:],
                in1=R[:].bitcast(i32),
                op0=AL.bitwise_and,
                op1=AL.bitwise_or,
            ).then_inc(s_v, 1)
            cnt[0] += 1
            n_dve[0] = cnt[0]

        @block.gpsimd
        def _(gpsimd):
            from concourse import library_config

            X3 = X.rearrange("p (g d) -> p g d", d=d)
            XV = dup_views(X3, XB)
            TwV = dup_views(Tw[:], Tw)
            TcV = dup_views(Tc[:], Tc)
            TxV = dup_views(Tx[:].rearrange("p (g d) -> p g d", d=d), Tx)

            # idx table: raw iota is p + 16*s; the HW gather reads index i
            # from partition i%16, col i//16 of EACH Q7 core's OWN
            # 16-partition block, so the table must be (p % 16) + 16*s
            # replicated — the DVE fold (t_off/t_ix) produces that from IW.
            if raw_iota:
                nc.gpsimd.iota(
                    IX[:], pattern=[[16, 8]], base=0, channel_multiplier=1
                ).then_inc(s_p, 1)
            else:
                nc.gpsimd.iota(
                    IW[:], pattern=[[16, 8]], base=0, channel_multiplier=1
                ).then_inc(s_p, 1)

            if not raw_iota and pool_fold:
                # finish the fold here: IW - (p & ~15) then convert to i16.
                # s_v=1 is the DVE AND (IO); both ops are tiny int32 column
                # ops the walrus accepts on Pool.
                gpsimd.wait_ge(s_v, 1)
                nc.gpsimd.tensor_tensor(
                    out=IW[:],
                    in0=IW[:],
                    in1=bass.AP(IO, 0, [IO[:].ap[0], [0, 8]]),
                    op=AL.subtract,
                ).then_inc(s_p, 1)
                gpsimd.wait_ge(s_p, 2)
                nc.gpsimd.tensor_copy(IX[:], IW[:]).then_inc(s_p, 1)

            # input: prepared gather, fired immediately
            nc.gpsimd.load_library(library_config.mlp)
            if raw_iota:
                gpsimd.wait_ge(s_p, 1)  # IX = iota (same engine)
            elif pool_fold:
                gpsimd.wait_ge(s_p, 3)  # IX finished above (same engine)
            else:
                gpsimd.wait_ge(s_v, 2)  # IX from the DVE fold
            out3 = bass.AP(XB, 0, [XB[:].ap[0], [FP, 1], [1, FP]])
            nc.gpsimd.dma_gather(
                out_ap=out3,
                in_ap=xb[:],
                idxs_ap=IX[:],
                num_idxs=128,
                num_idxs_reg=128,
                elem_size=FP,
                prepare_only=True,
                sem=s_in,
            ).then_inc(s_prep, 1)
            gpsimd.wait_ge(s_prep, 1)
            nc.gpsimd.trigger_dma(count=1)

            # dead-time setup under the standard library (memset matches the
            # boot-state usage in Bass.__init__): kv ctx idxs + O pad rows
            nc.gpsimd.load_library(library_config.standard)
            nc.gpsimd.memset(CI[:], 0).then_inc(s_p, 1)
            nc.gpsimd.memset(O[:], 0.0).then_inc(s_p, 1)
            p_ci = 4 if pool_fold else 2

            # output: prepared kv_writeback (batch=F, ncn=1 -> plain
            # (128, F) SBUF -> (F, 128) DRAM copy), fired after the result
            nc.gpsimd.load_library(library_config.attn)
            gpsimd.wait_ge(s_p, p_ci)  # CI (ctx idxs are read at prep time)
            O2 = O[:]
            in4 = bass.AP(O, O2.offset, [O2.ap[0], [F, 1], [1, F], [1, 1]])
            out4 = bass.AP(ys, 0, [[128, F], [1, 128], [1, 1], [1, 1]])
            nc.gpsimd.kv_writeback(
                out_ap=out4,
                in_ap=in4,
                ctx_idxs_ap=CI[:],
                prepare_only=True,
                sem=s_dma,
            ).then_inc(s_prep, 1)
            nc.gpsimd.load_library(library_config.standard)

            # LOO product (sign carrier) on Pool: add/sub/mult only here
            gpsimd.wait_ge(s_in, 16)
            nc.gpsimd.tensor_tensor(
                out=TwV(0, [[3, 2], [1, 3]]), in0=XV(0, [[0, 2], [2, 3]]),
                in1=XV(1, [[0, 2], [2, 3]]), op=AL.mult,
            ).then_inc(s_p, 1)
            gpsimd.wait_ge(s_p, p_ci + 2)
            nc.gpsimd.tensor_tensor(
                out=TcV(0, [[1, 3]]), in0=TwV(1, [[1, 3]]),
                in1=TwV(2, [[1, 3]]), op=AL.mult,
            ).then_inc(s_p, 1)
            gpsimd.wait_ge(s_p, p_ci + 3)
            nc.gpsimd.tensor_tensor(
                out=TxV(0, [[2, 3], [1, 2]]), in0=XV(1, [[2, 3], [-1, 2]]),
                in1=TcV(0, [[1, 3], [0, 2]]), op=AL.mult,
            ).then_inc(s_p, 1)

            # fire the prepared output DMA once O is in SBUF
            gpsimd.wait_ge(s_prep, 2)
            gpsimd.wait_ge(s_v, n_dve[0])
            nc.gpsimd.trigger_dma(count=1)
            # ring-drain fence on the same engine (same-engine sem: ~free)
            gpsimd.wait_ge(s_dma, 16)

        # manual block exit (see above): branch engines out, no barrier
        for engine, last_body in block.last_body.items():
            with nc.body(last_body, parent=nc.cur_bb, allow_existing_parent=True):
                engine.br(block.end_bb)
        nc.switch_bb(block.end_bb)
        nc.cur_block = None

    # Raw bass skips Bacc's codegen_inst_isa_subclasses pass; without it the
    # NEFF compiler sees empty .instr for extended-ISA ops ("ISA wrong length").
    mybir.codegen_inst_isa_subclasses(nc)
    return nc


def _uses_nc6(P, F, gpb, d):
    return d == 6 and P <= 128 and F <= 32


def _get_fast_nc(P, F, gpb, d):
    key = ("fast", P, F, gpb, d)
    if key not in _NC_CACHE:
        if _uses_nc6(P, F, gpb, d):
            _NC_CACHE[key] = _build_fast_nc6(P, F, gpb)
        else:
            _NC_CACHE[key] = _build_fast_nc(P, F, gpb, d)
    return _NC_CACHE[key]


def _run_spmd(nc, in_maps):
    global LAST_RESULT
    from concourse.bass_utils import run_bass_kernel_spmd

    res = run_bass_kernel_spmd(
        nc, in_maps, core_ids=list(range(NCORES)), trace=TRACE
    )
    LAST_RESULT = res
    return res.results


def _pack_xb(x, bias, slot_c, P, F):
    """Per-core xb for one core's (blocks, gpb, d) slot map. The nc6 path
    pads rows to 64 f32 (256B gather descriptors) and the tensor to 240
    rows (iota index upper bound)."""
    Bn = x.shape[0]
    e = slot_c
    valid = e >= 0
    ec = np.clip(e, 0, None)
    xs = np.where(valid[None], x[:, ec], np.float32(np.inf))
    bsv = np.where(valid, bias[0, ec], np.float32(0.0))
    bsv = np.broadcast_to(bsv[None], (Bn,) + bsv.shape)
    xbc = np.concatenate([xs.reshape(P, F), bsv.reshape(P, F)], axis=1)
    if _uses_nc6(P, F, None, slot_c.shape[-1]):
        pad = np.zeros((240, 64), np.float32)
        pad[:P, : 2 * F] = xbc
        xbc = pad
    return np.ascontiguousarray(xbc, np.float32)


def _kernel_fast(x, bias, leader):
    Bn, E_ = x.shape
    slot_all, d, blocks, gpb = _build_slots(leader, nbatch=Bn)
    P, F = Bn * blocks, gpb * d
    nc = _get_fast_nc(P, F, gpb, d)

    in_maps = []
    for c in range(NCORES):
        in_maps.append({"xb": _pack_xb(x, bias, slot_all[c], P, F)})

    results = _run_spmd(nc, in_maps)

    out = np.empty((Bn, E_), np.float32)
    for c in range(NCORES):
        e = slot_all[c]
        valid = e >= 0
        ys = results[c]["ys"]
        if ys.shape[0] != P:
            ys = ys.T  # (F, 128) layout from the kv_writeback path
        ys = ys[:P].reshape(Bn, blocks, gpb, d)
        out[:, e[valid]] = ys[:, valid]
    return out


def kernel(inputs, bias, inf_mask):
    x = np.ascontiguousarray(np.asarray(inputs), np.float32)
    bias = np.ascontiguousarray(np.asarray(bias), np.float32)
    inf_mask = np.asarray(inf_mask)

    leader = _analyze(inf_mask)
    if leader is not None:
        return _kernel_fast(x, bias, leader)
    return _kernel_dense(x, bias, inf_mask)


def _build_dense_nc(Bn, E, Ec):
    """Generic dense fallback: any (E, E) float mask, mask rows sharded
    per core (Ec rows, padded with +inf). Exactly follows the reference:
        nb    = mask == 0
        w     = nb ? x : 1       -> signs = sign(prod w)  (pairwise tree)
        mins  = min(|x| + mask)  (fused add+min reduce)
        out   = signs * max(mins - bias_row, 0)
    Output layout "ys" is (Ec, Bn) (row-major per output row; host transposes).
    """
    import contextlib

    import concourse.bass as bass
    from concourse import mybir

    f32 = mybir.dt.float32
    AL = mybir.AluOpType
    AX = mybir.AxisListType

    PT = 128
    ntiles = (Ec + PT - 1) // PT
    assert Ec % ntiles == 0 and (Ec // ntiles) <= PT
    TR = Ec // ntiles  # rows per tile

    nc = bass.Bass()
    mrows = nc.declare_dram_parameter("mrows", [Ec, E], f32, isOutput=False)
    xfull = nc.declare_dram_parameter("xfull", [Bn, E], f32, isOutput=False)
    brows = nc.declare_dram_parameter("brows", [Ec, 1], f32, isOutput=False)
    ys = nc.declare_dram_parameter("ys", [Ec, Bn], f32, isOutput=True)

    with contextlib.ExitStack() as ctx:
        XB = []
        for b in range(Bn):
            XB.append(ctx.enter_context(nc.sbuf_tensor(f"XBc{b}", [TR, E], f32)))
        MT = ctx.enter_context(nc.sbuf_tensor("MT", [TR, E], f32))
        W = ctx.enter_context(nc.sbuf_tensor("W", [TR, E], f32))
        SC = ctx.enter_context(nc.sbuf_tensor("SC", [TR, E], f32))
        SC2 = ctx.enter_context(nc.sbuf_tensor("SC2", [TR, E], f32))
        BC = ctx.enter_context(nc.sbuf_tensor("BC", [TR, 1], f32))
        MI = ctx.enter_context(nc.sbuf_tensor("MI", [TR, 1], f32))
        SG = ctx.enter_context(nc.sbuf_tensor("SG", [TR, 1], f32))
        PR = ctx.enter_context(nc.sbuf_tensor("PR", [TR, 1], f32))
        OT = ctx.enter_context(nc.sbuf_tensor("OT", [TR, Bn], f32))

        s_bc = ctx.enter_context(nc.semaphore("s_bc"))
        s_m = ctx.enter_context(nc.semaphore("s_m"))
        s_v = ctx.enter_context(nc.semaphore("s_v"))
        s_t = ctx.enter_context(nc.semaphore("s_t"))
        s_out = ctx.enter_context(nc.semaphore("s_out"))
        block = ctx.enter_context(nc.Block())

        @block.sync
        def _(sync):
            # broadcast each batch row of x across TR partitions (stride-0 AP)
            for b in range(Bn):
                src = bass.AP(xfull, b * E, [[0, TR], [1, E]])
                sync.dma_start(out=XB[b][:], in_=src).then_inc(s_bc, 16)
            for t in range(ntiles):
                if t:
                    # DVE done with tile t-1: MT/BC free, OT[t-1] complete
                    sync.wait_ge(s_t, t)
                    sync.dma_start(
                        out=ys[(t - 1) * TR : t * TR, :], in_=OT[:]
                    ).then_inc(s_out, 16)
                sync.dma_start(out=MT[:], in_=mrows[t * TR : (t + 1) * TR, :]).then_inc(s_m, 16)
                sync.dma_start(out=BC[:], in_=brows[t * TR : (t + 1) * TR, :]).then_inc(s_m, 16)
            sync.wait_ge(s_t, ntiles)
            sync.dma_start(
                out=ys[(ntiles - 1) * TR : ntiles * TR, :], in_=OT[:]
            ).then_inc(s_out, 16)
            sync.wait_ge(s_out, 16 * ntiles)

        @block.vector
        def _(vector):
            cnt = [0]
            waited = [0]

            def emit(fn, wait=None):
                if wait is None:
                    wait = cnt[0]
                if wait > waited[0]:
                    vector.wait_ge(s_v, wait)
                    waited[0] = wait
                fn().then_inc(s_v, 1)
                cnt[0] += 1
                return cnt[0]

            vector.wait_ge(s_bc, 16 * Bn)
            for t in range(ntiles):
                vector.wait_ge(s_m, 32 * (t + 1))
                if t:
                    # OT(t-1) out-DMA must have completed before rewriting OT
                    vector.wait_ge(s_out, 16 * t)
                # neighbor indicator for this tile's mask rows
                emit(lambda: nc.vector.tensor_single_scalar(out=W[:], in_=MT[:], scalar=0.0, op=AL.is_equal))
                for b in range(Bn):
                    # |x| for this batch into SC2
                    emit(lambda b=b: nc.vector.tensor_scalar_mul(SC2[:], XB[b][:], -1.0))
                    emit(lambda b=b: nc.vector.tensor_max(SC2[:], SC2[:], XB[b][:]))
                    # mins = reduce-min(mask + |x|)
                    emit(lambda: nc.vector.tensor_add(SC[:], MT[:], SC2[:]))
                    emit(lambda: nc.vector.tensor_reduce(
                        out=MI[:], in_=SC[:], axis=AX.X, op=AL.min))
                    # w = W * (x - 1) + 1  (= x where nb, else 1)
                    emit(lambda b=b: nc.vector.tensor_scalar_add(SC[:], XB[b][:], -1.0))
                    emit(lambda: nc.vector.tensor_mul(SC[:], W[:], SC[:]))
                    emit(lambda: nc.vector.tensor_scalar_add(SC[:], SC[:], 1.0))
                    # signs via pairwise product tree (reproduces fp underflow)
                    n = E
                    cur, other = SC, SC2
                    while n > 1:
                        h = n // 2
                        ce = cur[:, 0 : 2 * h].rearrange("p (h two) -> p h two", two=2)
                        emit(lambda ce=ce, other=other, h=h: nc.vector.tensor_tensor(
                            out=other[:, 0:h], in0=ce[:, :, 0:1], in1=ce[:, :, 1:2], op=AL.mult))
                        if n % 2:
                            emit(lambda cur=cur, other=other, n=n: nc.vector.tensor_mul(
                                other[:, 0:1], other[:, 0:1], cur[:, n - 1 : n]))
                        cur, other = other, cur
                        n = h
                    # SG = sign(prod) = is_gt - is_lt
                    emit(lambda cur=cur: nc.vector.tensor_single_scalar(out=SG[:], in_=cur[:, 0:1], scalar=0.0, op=AL.is_gt))
                    emit(lambda cur=cur: nc.vector.tensor_single_scalar(out=PR[:], in_=cur[:, 0:1], scalar=0.0, op=AL.is_lt))
                    emit(lambda: nc.vector.tensor_sub(SG[:], SG[:], PR[:]))
                    # out col = SG * max(mins - bias, 0)
                    emit(lambda: nc.vector.tensor_scalar(
                        out=MI[:], in0=MI[:], scalar1=BC[:], scalar2=0.0,
                        op0=AL.subtract, op1=AL.max))
                    emit(lambda b=b: nc.vector.tensor_mul(OT[:, b : b + 1], SG[:], MI[:]))
                vector.wait_ge(s_v, cnt[0])
                nc.vector.engine_nop().then_inc(s_t, 1)

    return nc


def _kernel_dense(x, bias, inf_mask):
    Bn, E = x.shape
    m = np.ascontiguousarray(np.asarray(inf_mask), np.float32)
    Ec = -(-E // NCORES)
    # round Ec up so it splits into <=128-row tiles evenly
    PT = 128
    ntiles = -(-Ec // PT)
    Ec = ntiles * PT if Ec > PT else Ec
    key = ("dense", Bn, E, Ec)
    if key not in _NC_CACHE:
        _NC_CACHE[key] = _build_dense_nc(Bn, E, Ec)
    nc = _NC_CACHE[key]

    in_maps = []
    for c in range(NCORES):
        lo = c * Ec
        rows = np.full((Ec, E), np.float32(np.inf), np.float32)
        bcol = np.zeros((Ec, 1), np.float32)
        hi = min(lo + Ec, E)
        if hi > lo:
            rows[: hi - lo] = m[lo:hi]
            bcol[: hi - lo, 0] = bias[0, lo:hi]
        in_maps.append(
            {
                "mrows": rows,
                "xfull": np.ascontiguousarray(x, np.float32),
                "brows": bcol,
            }
        )

    results = _run_spmd(nc, in_maps)

    out = np.empty((Bn, E), np.float32)
    for c in range(NCORES):
        lo = c * Ec
        hi = min(lo + Ec, E)
        if hi > lo:
            out[:, lo:hi] = results[c]["ys"][: hi - lo].T
    return out

